# revision 10
# baseline (speedup 1.0000x reference)
"""Circular convolution kernel for 8 TRN2 NeuronCores.

reference:  kv[j, k] = key[(j - k) mod m]   (m = n = 8192)
            result = kv @ input_x
            returns (result, kv)

Structure used: with rev[i] = key[(-i) mod m] and D2 = concat(rev, rev),
kv[j, k] = D2[m + k - j].  Row j is a contiguous slice of D2, so the whole
256 MB circulant materialization is pure (overlapped) data movement from a
64 KB buffer.  Sharding: rows j are split into 8 blocks of 1024 (one per
core).  Each core receives a small window W_c of D2 and builds in SBUF:

  Dsk[p, i] = W_c[i - p]          (skewed; row jl=128b+p of the output is
                                   Dsk[p, 1023-128b : 1023-128b+8192])
  Hc[p, c]  = W_c[p + 9087 - c]   (opposite skew; matmul tiles for the
                                   matvec are contiguous column slices)

The 8 row-block outputs are then plain contiguous SBUF->HBM DMAs (HBM
traffic is write-only, ~32 MB/core).  The matvec accumulates 64 fp32
matmuls [128,1]^T @ [128,512] per 512-row half into PSUM, overlapped with
the output DMAs.
"""
import sys

sys.path.insert(0, "/opt/trn_rl_repo")

import numpy as np
import bass_rust
import concourse.bass as bass
import concourse.tile as tile
from concourse import mybir
from concourse.bass_utils import run_bass_kernel_spmd

M = 8192
N_CORES = 8
ROWS = M // N_CORES          # 1024 rows per core
WLEN = ROWS - 1 + M          # 9215: per-core window of D2
SKW = 8064 + ROWS            # 9088: width of both skewed SBUF buffers


def _split_multi_waits(nc):
    """This walrus build accepts at most ONE sync wait per instruction,
    but Tile's sem assignment attaches several.  Hoist extra waits onto
    same-engine nops inserted immediately before the offending
    instruction (per-engine program order is preserved)."""
    for f in nc.m.functions:
        for bb in f.blocks:
            insts = bb.instructions
            i = 0
            while i < len(insts):
                ins = insts[i]
                si = ins.sync_info
                waits = list(si.on_wait) if si is not None and si.on_wait else []
                if len(waits) > 1:
                    si.on_wait = waits[-1:]
                    new = []
                    for w in waits[:-1]:
                        nop = nc.engines[ins.engine].nop()
                        nop.ins.sync_info = bass_rust.SyncInfo(
                            on_wait=[w], on_update=[]
                        )
                        new.append(nop.ins)
                    new_names = {n_.name for n_ in new}
                    for bb2 in f.blocks:
                        l2 = bb2.instructions
                        for j in range(len(l2) - 1, -1, -1):
                            if l2[j].name in new_names:
                                l2.pop(j)
                    for k, n_ in enumerate(new):
                        insts.insert(i + k, n_)
                    i += len(new)
                i += 1


def _build_nc():
    nc = bass.Bass()
    f32 = mybir.dt.float32
    d2win = nc.declare_dram_parameter("d2win", [WLEN], f32, isOutput=False)
    xrow = nc.declare_dram_parameter("xrow", [1, M], f32, isOutput=False)
    ones_r = nc.declare_dram_parameter("ones_r", [1, 128], f32, isOutput=False)
    kv_out = nc.declare_dram_parameter("kv_out", [ROWS, M], f32, isOutput=True)
    res_out = nc.declare_dram_parameter("res_out", [128, 8], f32, isOutput=True)

    with tile.TileContext(nc) as tc:
        with tc.tile_pool(name="sb", bufs=1) as pool, \
             tc.tile_pool(name="scr", bufs=2) as scr_pool, \
             tc.tile_pool(name="scr2", bufs=1) as scr2_pool, \
             tc.tile_pool(name="ps", bufs=2, space="PSUM") as psum_pool:
            # small loads go on the scalar HWDGE ring so they don't queue
            # behind the bulk transfers on the sync ring
            x_sb = pool.tile([1, M], f32)
            nc.scalar.dma_start(x_sb[:], xrow[:])
            ones_sb = pool.tile([1, 128], f32)
            nc.scalar.dma_start(ones_sb[:], ones_r[:])

            # broadcast x across partitions via K=1 outer product:
            # psum[p, n] = ones[0, p] * x[0, n]
            xbc = pool.tile([128, M], f32)
            for jj in range(16):
                psb = psum_pool.tile([128, 512], f32)
                nc.tensor.matmul(
                    psb[:],
                    ones_sb[0:1, :],
                    x_sb[0:1, 512 * jj : 512 * (jj + 1)],
                    start=True,
                    stop=True,
                )
                nc.vector.tensor_copy(xbc[:, 512 * jj : 512 * (jj + 1)], psb[:])

            # dsk[p, i] = W_c[p + i]; partition p holds row 128b+127-p of
            # block b at cols [896-128b, 896-128b+M)
            dsk = pool.tile([128, SKW], f32)
            nc.sync.dma_start(
                dsk[:], bass.AP(d2win, 0, [[1, 128], [1, SKW]])
            )

            # matvec reading dsk directly: DVE elementwise mult, then a
            # free-axis reduce split between ACT (copy-with-accumulate) and
            # DVE (tensor_reduce) so both engines finish early.
            # res_sb[p, b] = sum_k dsk[p, off_b + k] * x[k]
            #             = result[j0 + 128b + 127 - p]
            res_sb = pool.tile([128, 8], f32)
            for b in range(8):
                off = 896 - 128 * b
                scratch = scr_pool.tile([128, M], f32)
                nc.vector.tensor_tensor(
                    out=scratch[:],
                    in0=dsk[:, off : off + M],
                    in1=xbc[:],
                    op=mybir.AluOpType.mult,
                )
                if b % 4 == 3:
                    nc.vector.tensor_reduce(
                        res_sb[:, b : b + 1],
                        scratch[:],
                        axis=mybir.AxisListType.X,
                        op=mybir.AluOpType.add,
                    )
                else:
                    scratch2 = scr2_pool.tile([128, M], mybir.dt.bfloat16)
                    nc.scalar.activation(
                        scratch2[:],
                        scratch[:],
                        mybir.ActivationFunctionType.Copy,
                        accum_out=res_sb[:, b : b + 1],
                    )

            # 8 row-block writes.  Partition p carries row jl = 128b+127-p;
            # negative DRAM row steps are rejected by the verifier, so the
            # kernel stores the core's rows in REVERSED order
            # (kv_out[r] = kv[j0 + 1023 - r]) and the host flips at gather.
            for b in range(8):
                nc.sync.dma_start(
                    kv_out[128 * (7 - b) : 128 * (8 - b), :],
                    dsk[:, 896 - 128 * b : 896 - 128 * b + M],
                )

            nc.sync.dma_start(res_out[:], res_sb[:])

    _split_multi_waits(nc)
    return nc


_NC_CACHE = None


def _get_nc():
    global _NC_CACHE
    if _NC_CACHE is None:
        _NC_CACHE = _build_nc()
    return _NC_CACHE


def _prep_in_maps(key, input_x):
    key = np.asarray(key, dtype=np.float32).reshape(M)
    x = np.asarray(input_x, dtype=np.float32).reshape(M)

    # rev[i] = key[(-i) mod m]; D2 = concat(rev, rev); kv[j,k] = D2[m+k-j]
    rev = key[(-np.arange(M)) % M]
    d2 = np.concatenate([rev, rev])

    xrow = np.ascontiguousarray(x.reshape(1, M))
    ones_r = np.ones((1, 128), np.float32)

    in_maps = []
    for c in range(N_CORES):
        j0 = ROWS * c
        w = d2[M - j0 - (ROWS - 1) : M - j0 + M]  # length WLEN
        in_maps.append(
            {"d2win": np.ascontiguousarray(w), "xrow": xrow, "ones_r": ones_r}
        )
    return in_maps


def _run(key, input_x, trace=False):
    nc = _get_nc()
    in_maps = _prep_in_maps(key, input_x)
    out = run_bass_kernel_spmd(
        nc, in_maps, core_ids=list(range(N_CORES)), trace=trace
    )
    results = out.results
    kv = np.concatenate(
        [results[c]["kv_out"][::-1] for c in range(N_CORES)], axis=0
    )
    # res_out[p, b] = result[j0 + 128b + 127 - p]
    res = np.concatenate(
        [results[c]["res_out"][::-1].T.ravel() for c in range(N_CORES)]
    )
    return (res, kv), out


def kernel(key, input_x, factor=None, **_unused):
    (res, kv), _ = _run(key, input_x, trace=False)
    return res, kv


# revision 11
# speedup vs baseline: 1.2168x; 1.2168x over previous
"""Circular convolution kernel for 8 TRN2 NeuronCores.

reference:  kv[j, k] = key[(j - k) mod m]   (m = n = 8192)
            result = kv @ input_x
            returns (result, kv)

Structure used: with rev[i] = key[(-i) mod m] and D2 = concat(rev, rev),
kv[j, k] = D2[m + k - j].  Row j is a contiguous slice of D2, so the whole
256 MB circulant materialization is pure (overlapped) data movement from a
64 KB buffer.  Sharding: rows j are split into 8 blocks of 1024 (one per
core).  Each core receives a small window W_c of D2 (and its reversal) and
builds two skewed SBUF buffers with one overlapped-read DMA each:

  dsk[p, i] = W_c[p + i]        row 128b+127-p of output block b is
                                dsk[p, 896-128b : 896-128b+8192]
  hc[q, c]  = W_c[9214 - q - c] matmul lhsT tiles for the matvec are
                                contiguous 128-col slices of hc

The 8 row-block outputs are plain contiguous SBUF->HBM DMAs (HBM traffic
is write-only, ~32 MB/core, the roofline term).  Rows inside each block
come out reversed (the BIR verifier rejects negative partition/row steps),
so the host flips row order during the gather.

The matvec uses the block-Toeplitz diagonal structure: for diagonal
d = kc - jt_l in [-7, 63] one 128-wide hc slice is the stationary matmul
operand shared by all 8 j-tiles of the core:

  out[jp, jt_l] += hc[:, 8064-128d : +128]^T @ xpad[:, 7+d : +8]

(x reshaped to [128, 64], rows flipped to match hc's skew, zero-padded by
7 columns on each side so out-of-range kc contributes 0).  71 fp32
matmuls accumulate in one PSUM tile, fully overlapped with the DMA phase.
"""
import sys

sys.path.insert(0, "/opt/trn_rl_repo")

import numpy as np
import bass_rust
import concourse.bass as bass
import concourse.tile as tile
from concourse import mybir
from concourse.bass_utils import run_bass_kernel_spmd

M = 8192
N_CORES = 8
ROWS = M // N_CORES          # 1024 rows per core
WLEN = ROWS - 1 + M          # 9215: per-core window of D2
SKW = 8064 + ROWS            # 9088: width of both skewed SBUF buffers


def _split_multi_waits(nc):
    """This walrus build accepts at most ONE sync wait per instruction,
    but Tile's sem assignment attaches several.  Hoist extra waits onto
    same-engine nops inserted immediately before the offending
    instruction (per-engine program order is preserved, and the nop
    executes its wait first, so semantics are identical)."""
    for f in nc.m.functions:
        for bb in f.blocks:
            insts = bb.instructions
            i = 0
            while i < len(insts):
                ins = insts[i]
                si = ins.sync_info
                waits = list(si.on_wait) if si is not None and si.on_wait else []
                if len(waits) > 1:
                    si.on_wait = waits[-1:]
                    new = []
                    for w in waits[:-1]:
                        nop = nc.engines[ins.engine].nop()
                        nop.ins.sync_info = bass_rust.SyncInfo(
                            on_wait=[w], on_update=[]
                        )
                        new.append(nop.ins)
                    new_names = {n_.name for n_ in new}
                    for bb2 in f.blocks:
                        l2 = bb2.instructions
                        for j in range(len(l2) - 1, -1, -1):
                            if l2[j].name in new_names:
                                l2.pop(j)
                    for k, n_ in enumerate(new):
                        insts.insert(i + k, n_)
                    i += len(new)
                i += 1


def _build_nc():
    nc = bass.Bass()
    f32 = mybir.dt.float32
    d2win = nc.declare_dram_parameter("d2win", [WLEN], f32, isOutput=False)
    d2winR = nc.declare_dram_parameter("d2winR", [WLEN], f32, isOutput=False)
    x2d = nc.declare_dram_parameter("x2d", [128, 78], f32, isOutput=False)
    kv_out = nc.declare_dram_parameter("kv_out", [ROWS, M], f32, isOutput=True)
    res_out = nc.declare_dram_parameter("res_out", [128, 8], f32, isOutput=True)

    with tile.TileContext(nc) as tc:
        with tc.tile_pool(name="sb", bufs=1) as pool, \
             tc.tile_pool(name="ps", bufs=2, space="PSUM") as psum_pool:
            # dsk[p, i] = W_c[p + i]
            dsk = pool.tile([128, SKW], f32)
            nc.sync.dma_start(
                dsk[:], bass.AP(d2win, 0, [[1, 128], [1, SKW]])
            )
            # hc[q, c] = W_c[9214 - q - c] = W_c_reversed[q + c]
            hc = pool.tile([128, SKW], f32)
            nc.sync.dma_start(
                hc[:], bass.AP(d2winR, 0, [[1, 128], [1, SKW]])
            )
            # xpad on the scalar HWDGE ring so it doesn't queue behind the
            # bulk transfers on the sync ring
            xt = pool.tile([128, 78], f32)
            nc.scalar.dma_start(xt[:], x2d[:])

            # 8 row-block writes (row order inside a block is reversed;
            # host flips at gather)
            for b in range(8):
                nc.sync.dma_start(
                    kv_out[128 * (7 - b) : 128 * (8 - b), :],
                    dsk[:, 896 - 128 * b : 896 - 128 * b + M],
                )

            # matvec via block-Toeplitz diagonals
            ps = psum_pool.tile([128, 8], f32)
            for di, d in enumerate(range(-7, 64)):
                nc.tensor.matmul(
                    ps[:],
                    hc[:, 8064 - 128 * d : 8064 - 128 * d + 128],
                    xt[:, 7 + d : 7 + d + 8],
                    start=(di == 0),
                    stop=(di == 70),
                )
            # contiguous [128, 8] store; host transposes the 4 KB result
            # (strided 32 B writes would cost ~26 us of SDMA RMW)
            res_sb = pool.tile([128, 8], f32)
            nc.vector.tensor_copy(res_sb[:], ps[:])
            nc.sync.dma_start(res_out[:], res_sb[:])

    _split_multi_waits(nc)
    return nc


_NC_CACHE = None


def _get_nc():
    global _NC_CACHE
    if _NC_CACHE is None:
        _NC_CACHE = _build_nc()
    return _NC_CACHE


def _prep_in_maps(key, input_x):
    key = np.asarray(key, dtype=np.float32).reshape(M)
    x = np.asarray(input_x, dtype=np.float32).reshape(M)

    # rev[i] = key[(-i) mod m]; D2 = concat(rev, rev); kv[j,k] = D2[m+k-j]
    rev = key[(-np.arange(M)) % M]
    d2 = np.concatenate([rev, rev])

    # x2[q, kc] = x[128kc + (127-q)] (rows flipped to match hc's skew),
    # zero-padded by 7 columns on each side for the diagonal scheme
    x2 = np.zeros((128, 78), np.float32)
    x2[:, 7:71] = x.reshape(64, 128).T[::-1]

    in_maps = []
    for c in range(N_CORES):
        j0 = ROWS * c
        w = d2[M - j0 - (ROWS - 1) : M - j0 + M]  # length WLEN
        in_maps.append(
            {
                "d2win": np.ascontiguousarray(w),
                "d2winR": np.ascontiguousarray(w[::-1]),
                "x2d": x2,
            }
        )
    return in_maps


def _run(key, input_x, trace=False):
    nc = _get_nc()
    in_maps = _prep_in_maps(key, input_x)
    out = run_bass_kernel_spmd(
        nc, in_maps, core_ids=list(range(N_CORES)), trace=trace
    )
    results = out.results
    # rows inside each core's block are stored reversed
    kv = np.concatenate(
        [results[c]["kv_out"][::-1] for c in range(N_CORES)], axis=0
    )
    # res_out[jp, jt_l] = result[j0 + 128*jt_l + jp]
    res = np.concatenate(
        [results[c]["res_out"].T.ravel() for c in range(N_CORES)]
    )
    return (res, kv), out


def kernel(key, input_x, factor=None, **_unused):
    (res, kv), _ = _run(key, input_x, trace=False)
    return res, kv
